# revision 21
# baseline (speedup 1.0000x reference)
"""DVAE GNN message-passing kernel for 8 Trainium2 NeuronCores.

Data parallel over batch B=2048 -> 256 graphs/core (2 tiles of 128).
Each core runs the full 20-step topological scan, weights replicated.

Math (per sample b, step v in 0..19, Hfwd starts at 0):
  gated_u = sigmoid(Wg @ [H_u, vid_u] + bg) * (Wm @ [H_u, vid_u])
  Hin_v   = sum_{u<v} adj[b,u,v] * gated_u + const_v    (const_v is the
            u>=v part where H_u = 0 -- precomputed on host)
  H_v     = GRUCell(x_v, Hin_v)
  mu,lv   = W1 @ H_19 + b1, W2 @ H_19 + b2

Device schedule (per step): the adj-weighted message chains run on DVE as
fused scalar_tensor_tensor MACs with f32 accumulators seeded by the host
const tables, load-leveled across steps with an EDF prefill schedule
(future steps' chains accumulate early, during the current step's matmul
phases).  x-side r/z contributions + biases enter via a K=48 one-hot
matmul; the n-gate x-side (i_n + b_in, which the r-gate must NOT
multiply) comes from a host table added after the r*h_n product.  The
vertex-id one-hot enters via a per-step DMA update of the gated weights'
bias row.  PSUM: 8 banks = 2 transpose staging + 3 per batch tile
(r/z/n groups, reused by the gated zp/mp matmuls).
"""

import sys
import numpy as np

for _p in ("/opt/trn_rl_repo",):
    if _p not in sys.path:
        sys.path.insert(0, _p)

B, MAXN, NVT, HS, NZ = 2048, 20, 26, 501, 56
HS2 = HS + 1                  # 502: per-gate column pitch; col 501 = ones
NVT_EFF = NVT + MAXN          # 46
XDIM = NVT_EFF + 1            # 47
XR = XDIM + 1                 # 48: x rows incl ones row
NCORES = 8
BS = B // NCORES              # 256 samples per core
RZ = 2 * HS

# Haug^T chunks (501 hidden rows + ones row at 501)
CH = [(0, 128), (128, 128), (256, 128), (384, 118)]

CHAIN_CAP = 10   # prefill MAC budget per tile per step
LOOKAHEAD = 8    # chains w <= v + LOOKAHEAD may be prefilled at step v
AB_SLOTS = 8     # ring size for in-flight chain accumulators


def _chain_schedule():
    """EDF schedule for partial chain MACs (terms u <= w-2 of chain w).

    The final term u = w-1 runs as a fused stt right after G_{w-1} is
    produced at step w-1.  Returns sched[v] = [(w, u), ...] per step.
    """
    pend = [(w, u) for w in range(1, MAXN) for u in range(w - 1)]
    done = set()
    sched = [[] for _ in range(MAXN)]
    for v in range(MAXN):
        budget = CHAIN_CAP
        elig = [p for p in pend if p not in done and p[1] <= v - 1
                and p[0] - 1 >= v and p[0] <= v + LOOKAHEAD]
        elig.sort()
        for p in elig:
            mand = p[0] == v + 1
            if budget <= 0 and not mand:
                continue
            sched[v].append(p)
            done.add(p)
            budget -= 1
    assert len(done) == len(pend), (len(done), len(pend))
    for w in range(1, MAXN):
        for u in range(w - 1):
            vdone = next(v for v in range(MAXN) if (w, u) in sched[v])
            assert vdone <= w - 1
    return sched


def _pack_layout():
    """Column layout (fp32 elements) of the packed static tensor."""
    ents = {}
    col = 0

    def put(name, nrows, ncols):
        nonlocal col
        ents[name] = (nrows, col, ncols)
        col += ncols

    put("wrzx", XR, 2 * HS2)
    for i, (o, s) in enumerate(CH):
        put(f"wrzh{i}", s, 2 * HS2)
    for i, (o, s) in enumerate(CH):
        put(f"whn{i}", s, HS2)
    for i, (o, s) in enumerate(CH):
        put(f"wg{i}", s, HS2)
    for i, (o, s) in enumerate(CH):
        put(f"wm{i}", s, HS2)
    put("pk", XR, MAXN * BS)
    for i, (o, s) in enumerate(CH):
        put(f"w12{i}", s, 2 * NZ)
    return ents, col


FIXCOLS = 128 + 2 * MAXN * MAXN   # ident + adjg0 + adjg1 (f32)


_PROG = None


def _build_program():
    import concourse.tile as tile
    from concourse import bacc, mybir

    f32 = mybir.dt.float32
    mdt = mybir.dt.float32r
    bf16 = mybir.dt.bfloat16
    AF = mybir.ActivationFunctionType
    OP = mybir.AluOpType

    nc = bacc.Bacc("TRN2", target_bir_lowering=False, debug=False)

    ents, ncols = _pack_layout()
    d_wpack = nc.dram_tensor("wpack", [128, ncols], bf16,
                             kind="ExternalInput").ap()
    d_wfix = nc.dram_tensor("wfix", [128, FIXCOLS], f32,
                            kind="ExternalInput").ap()
    d_const = nc.dram_tensor("constt", [MAXN * 2 * 128, HS2], f32,
                             kind="ExternalInput").ap()
    d_gin = nc.dram_tensor("gint", [MAXN * 2 * 128, HS2], bf16,
                           kind="ExternalInput").ap()
    d_wgvb = nc.dram_tensor("wgvb", [MAXN, HS2], bf16,
                            kind="ExternalInput").ap()
    d_wmvb = nc.dram_tensor("wmvb", [MAXN, HS2], bf16,
                            kind="ExternalInput").ap()
    d_out = nc.dram_tensor("out", [BS, 2 * NZ], f32, kind="ExternalOutput").ap()

    sched = _chain_schedule()
    # first step at which chain w's accumulator opens (first partial MAC)
    first = {}
    for v in range(MAXN):
        for (w, u) in sched[v]:
            first.setdefault(w, v)
    first.setdefault(1, 0)   # chain 1 has no partials; cst read at GM(0)

    def mm(out, lhsT, rhs, start, stop):
        nc.tensor.matmul(out, lhsT, rhs, start=start, stop=stop)

    with tile.TileContext(nc) as tc:
        with (
            tc.tile_pool(name="statics", bufs=1) as sp,
            tc.tile_pool(name="gstore", bufs=2 * (MAXN - 1)) as gp,
            tc.tile_pool(name="csts", bufs=4) as cp,
            tc.tile_pool(name="gins", bufs=2) as gip,
            tc.tile_pool(name="hint", bufs=1) as hip,
            tc.tile_pool(name="ht", bufs=1) as htp,
            tc.tile_pool(name="work", bufs=1) as wp,
            tc.tile_pool(name="psum", bufs=1, space="PSUM") as pp,
        ):
            WPACK = sp.tile([128, ncols], bf16, tag="wpack", name="wpack")
            WFIX = sp.tile([128, FIXCOLS], f32, tag="wfix", name="wfix")
            nc.sync.dma_start(WFIX[:, :], d_wfix)
            NSPLIT = 12
            cuts = [ncols * i // NSPLIT for i in range(NSPLIT + 1)]
            for c0, c1 in zip(cuts[:-1], cuts[1:]):
                nc.sync.dma_start(WPACK[:, c0:c1], d_wpack[:, c0:c1])

            def sl(name, dt=None):
                nr, c0, ncl = ents[name]
                ap = WPACK[0:nr, c0:c0 + ncl]
                return ap.bitcast(dt) if dt else ap

            PK = sl("pk")
            WRZH = [sl(f"wrzh{i}") for i in range(4)]
            WHN = [sl(f"whn{i}") for i in range(4)]
            WRZX = sl("wrzx")
            WG = [sl(f"wg{i}") for i in range(4)]
            WM = [sl(f"wm{i}") for i in range(4)]
            W12 = [sl(f"w12{i}") for i in range(4)]
            IDN = WFIX[:, 0:128]
            ADJG = [WFIX[:, 128 + t * MAXN * MAXN:128 + (t + 1) * MAXN * MAXN]
                    for t in range(2)]

            # gated message vectors, one per (vertex, batch-tile)
            Gt = [[gp.tile([128, HS2], bf16, tag="g", name=f"g{u}_{t}")
                   for t in range(2)] for u in range(MAXN - 1)]

            cst, gin = {}, {}

            def dma_cst(w):
                for t in range(2):
                    c = cp.tile([128, HS2], f32, tag=f"cst{t}", name=f"cst{w}_{t}")
                    nc.sync.dma_start(c[:, :], d_const[(w * 2 + t) * 128:
                                                       (w * 2 + t + 1) * 128, :])
                    cst[(w, t)] = c

            def dma_gin(v):
                for t in range(2):
                    g = gip.tile([128, HS2], bf16, tag=f"gin{t}", name=f"gin{v}_{t}")
                    nc.sync.dma_start(g[:, :], d_gin[(v * 2 + t) * 128:
                                                     (v * 2 + t + 1) * 128, :])
                    gin[(v, t)] = g

            dma_cst(0)
            dma_cst(1)
            for w in sorted(first):
                if first[w] <= 1 and w > 1:
                    dma_cst(w)
            dma_gin(0)
            dma_gin(1)

            # chain accumulators: ab[(w, t)] is the running f32 partial sum
            # of chain w (const folded into the first MAC)
            ab = {}
            acc = {}           # (w, t) -> finished Hin tile
            for t in range(2):
                acc[(0, t)] = cst[(0, t)]

            def emit_macs(v, terms, offload=False):
                for idx, (w, u) in enumerate(terms):
                    for t in range(2):
                        off = t == 1
                        sc = ADJG[t][:, u * MAXN + w:u * MAXN + w + 1]
                        if (w, t) not in ab:
                            a = wp.tile([128, HS2], bf16, tag=f"ab{t}",
                                        bufs=AB_SLOTS, name=f"ab{w}_{t}")
                            ab[(w, t)] = a
                            srct = cst[(w, t)]
                        else:
                            a = ab[(w, t)]
                            srct = a
                        if off:
                            sm = wp.tile([128, HS2], bf16, tag=f"sm{t}",
                                         bufs=3, name=f"sm{w}_{u}_{t}")
                            nc.scalar.activation(sm[:, :], Gt[u][t][:, :],
                                                 AF.Copy, scale=sc)
                            nc.vector.tensor_tensor(a[:, :], srct[:, :],
                                                    sm[:, :], OP.add)
                        else:
                            nc.vector.scalar_tensor_tensor(
                                a[:, :], Gt[u][t][:, :], sc, srct[:, :],
                                OP.mult, OP.add)

            HINT = [None, None]
            HT = [None, None]
            htile = [None, None]
            psr = [None, None]
            psz = [None, None]
            psn = [None, None]

            def emit_tin(v, t):
                stage = pp.tile([128, 512], f32, tag="st", bufs=2,
                                name=f"sti{v}_{t}")
                a = acc[(v, t)]
                for i, (o, w) in enumerate(CH):
                    nc.tensor.transpose(
                        stage[0:w, 128 * i:128 * i + 128],
                        a[:, o:o + w], IDN[:, :])
                hi = hip.tile([128, 512], bf16, tag=f"hint{t}", name=f"hint{v}_{t}")
                nc.scalar.copy(hi[:, :], stage[:, :])
                HINT[t] = hi

            def emit_mm(v, t):
                xsl = PK[0:XR, v * BS + t * 128:v * BS + (t + 1) * 128]
                pr = pp.tile([128, 512], f32, tag=f"pa{t}", name=f"pr{v}_{t}")
                pz = pp.tile([128, 512], f32, tag=f"pb{t}", name=f"pz{v}_{t}")
                pn = pp.tile([128, 512], f32, tag=f"pc{t}", name=f"pn{v}_{t}")
                hi = HINT[t]
                mm(pr[:, 0:HS2], xsl, WRZX[:, 0:HS2], True, False)
                for i, (o, w) in enumerate(CH):
                    mm(pr[:, 0:HS2], hi[0:w, 128 * i:128 * i + 128],
                       WRZH[i][:, 0:HS2], False, i == 3)
                mm(pz[:, 0:HS2], xsl, WRZX[:, HS2:2 * HS2], True, False)
                for i, (o, w) in enumerate(CH):
                    mm(pz[:, 0:HS2], hi[0:w, 128 * i:128 * i + 128],
                       WRZH[i][:, HS2:2 * HS2], False, i == 3)
                for i, (o, w) in enumerate(CH):
                    mm(pn[:, 0:HS2], hi[0:w, 128 * i:128 * i + 128],
                       WHN[i][:, 0:HS2], i == 0, i == 3)
                psr[t], psz[t], psn[t] = pr, pz, pn

            def emit_act(v, t):
                r = wp.tile([128, HS2], bf16, tag=f"r{t}", name=f"r{v}_{t}")
                z = wp.tile([128, HS2], bf16, tag=f"z{t}", name=f"z{v}_{t}")
                nc.scalar.activation(r[:, :], psr[t][:, 0:HS2], AF.Sigmoid)
                nc.scalar.activation(z[:, :], psz[t][:, 0:HS2], AF.Sigmoid)
                tmp = wp.tile([128, HS], f32, tag=f"tmp{t}", name=f"tmp{v}_{t}")
                # n = tanh(r * h_n + (i_n + b_in));  h_n (incl b_hn) in PSUM
                nc.vector.tensor_tensor(tmp[:, :], r[:, 0:HS], psn[t][:, 0:HS],
                                        OP.mult)
                nc.gpsimd.tensor_tensor(tmp[:, :], tmp[:, :],
                                        gin[(v, t)][:, 0:HS], OP.add)
                n = wp.tile([128, HS], bf16, tag=f"n{t}", name=f"n{v}_{t}")
                nc.scalar.activation(n[:, :], tmp[:, :], AF.Tanh)
                a = acc[(v, t)]
                d = wp.tile([128, HS], f32, tag=f"d{t}", name=f"d{v}_{t}")
                nc.gpsimd.tensor_tensor(d[:, :], a[:, 0:HS], n[:, :], OP.subtract)
                h = wp.tile([128, HS2], f32, tag=f"h{t}", bufs=2,
                            name=f"h{v}_{t}")
                nc.gpsimd.tensor_tensor(h[:, 0:HS], d[:, :], z[:, 0:HS], OP.mult)
                nc.gpsimd.tensor_tensor(h[:, 0:HS], h[:, 0:HS], n[:, :], OP.add)
                if v < 2:
                    nc.gpsimd.memset(h[:, HS:HS2], 1.0)  # ones col -> bias row
                htile[t] = h

            def emit_th(v, t):
                stage = pp.tile([128, 512], f32, tag="st", bufs=2,
                                name=f"sth{v}_{t}")
                h = htile[t]
                for i, (o, w) in enumerate(CH):
                    nc.tensor.transpose(
                        stage[0:w, 128 * i:128 * i + 128],
                        h[:, o:o + w], IDN[:, :])
                ht = htp.tile([128, 512], bf16, tag=f"ht{t}", name=f"ht{v}_{t}")
                nc.scalar.copy(ht[:, :], stage[:, :])
                HT[t] = ht

            def emit_gm(v, t):
                zp = pp.tile([128, 512], f32, tag=f"pa{t}", name=f"zp{v}_{t}")
                mp = pp.tile([128, 512], f32, tag=f"pb{t}", name=f"mp{v}_{t}")
                ht = HT[t]
                for i, (o, w) in enumerate(CH):
                    mm(zp[:, 0:HS2], ht[0:w, 128 * i:128 * i + 128],
                       WG[i][:, 0:HS2], i == 0, i == 3)
                for i, (o, w) in enumerate(CH):
                    mm(mp[:, 0:HS2], ht[0:w, 128 * i:128 * i + 128],
                       WM[i][:, 0:HS2], i == 0, i == 3)
                sg = wp.tile([128, HS2], bf16, tag=f"sg{t}", name=f"sg{v}_{t}")
                nc.scalar.activation(sg[:, :], zp[:, 0:HS2], AF.Sigmoid)
                g = Gt[v][t]
                nc.vector.tensor_tensor(g[:, :], sg[:, :], mp[:, 0:HS2], OP.mult)
                # finish chain v+1: acc = a_{v,v+1} * G_v + partial
                w1 = v + 1
                sc = ADJG[t][:, v * MAXN + w1:v * MAXN + w1 + 1]
                src = ab.get((w1, t), cst.get((w1, t)))
                at = wp.tile([128, HS2], f32, tag=f"acc{t}", bufs=2,
                             name=f"acc{w1}_{t}")
                nc.vector.scalar_tensor_tensor(at[:, :], g[:, :], sc,
                                               src[:, :], OP.mult, OP.add)
                acc[(w1, t)] = at

            for v in range(MAXN):
                fin = [p for p in sched[v] if p[0] == v + 1]
                pre = [p for p in sched[v] if p[0] != v + 1]
                emit_macs(v, fin)

                emit_tin(v, 0)
                emit_tin(v, 1)
                emit_mm(v, 0)
                emit_mm(v, 1)
                emit_act(v, 0)
                emit_th(v, 0)
                emit_act(v, 1)
                if v < MAXN - 1:
                    emit_gm(v, 0)
                    emit_th(v, 1)
                    emit_gm(v, 1)
                    emit_macs(v, pre)
                else:
                    emit_th(v, 1)
                if v < MAXN - 1:
                    # gated bias+vid row update for step v+1
                    nc.sync.dma_start(WG[3][117:118, 0:HS2],
                                      d_wgvb[v + 1:v + 2, :])
                    nc.sync.dma_start(WM[3][117:118, 0:HS2],
                                      d_wmvb[v + 1:v + 2, :])
                # prefetch streams for upcoming steps
                for w in sorted(first):
                    if first[w] == v + 1 and (w, 0) not in cst:
                        dma_cst(w)
                if v + 2 < MAXN:
                    dma_gin(v + 2)

            # readout from HT (H_19)
            for t in range(2):
                op = pp.tile([128, 512], f32, tag=f"pc{t}", name=f"op{t}")
                ht = HT[t]
                for i, (o, w) in enumerate(CH):
                    mm(op[:, 0:2 * NZ], ht[0:w, 128 * i:128 * i + 128],
                       W12[i][:, :], i == 0, i == 3)
                ob = wp.tile([128, 2 * NZ], f32, tag=f"ob{t}", name=f"ob{t}")
                nc.scalar.copy(ob[:, :], op[:, 0:2 * NZ])
                nc.sync.dma_start(d_out[t * 128:(t + 1) * 128, :], ob[:, :])

    nc.compile()
    return nc


def _host_prep(types, feats, adj, Wg, bg, Wm, W_ih, b_ih, W_hh, b_hh, W1, b1,
               W2, b2):
    f = np.float32
    types = np.asarray(types).astype(np.int64)
    feats = np.asarray(feats, dtype=f)
    adj = np.asarray(adj, dtype=f)
    Wg, bg, Wm = np.asarray(Wg, f), np.asarray(bg, f), np.asarray(Wm, f)
    W_ih, b_ih = np.asarray(W_ih, f), np.asarray(b_ih, f)
    W_hh, b_hh = np.asarray(W_hh, f), np.asarray(b_hh, f)
    W1, b1 = np.asarray(W1, f), np.asarray(b1, f)
    W2, b2 = np.asarray(W2, f), np.asarray(b2, f)

    bsz = types.shape[0]
    bs = bsz // NCORES

    # X^T with ones row: [48, MAXN*bs] slices per core
    X = np.zeros((bsz, MAXN, XR), dtype=f)
    onehot = np.eye(NVT_EFF, dtype=f)[types.reshape(-1) % NVT_EFF]
    X[:, :, :NVT_EFF] = onehot.reshape(bsz, MAXN, NVT_EFF)
    X[:, :, NVT_EFF] = feats
    X[:, :, XDIM] = 1.0

    # constant gated vectors c_u for zero hidden state
    zg = 1.0 / (1.0 + np.exp(-(bg[None, :] + Wg[:, HS:].T)))   # [20, 501]
    C = (zg * Wm[:, HS:].T).astype(f)
    umask = (np.arange(MAXN)[:, None] >= np.arange(MAXN)[None, :]).astype(f)
    const = np.einsum('buv,uh->bvh', adj * umask[None, :, :], C).astype(f)

    # i_n + b_in per (b, v): one-hot gather instead of a matmul
    Wn = W_ih[RZ:]                                   # [501, 47]
    ginb = Wn.T[types.reshape(-1) % NVT_EFF]         # [B*20, 501] type rows
    ginb = ginb.reshape(bsz, MAXN, HS) + feats[..., None] * Wn[:, NVT_EFF]
    ginb = (ginb + b_ih[RZ:]).astype(f)              # [B, 20, 501]

    def padg(a):            # [rows, HS] -> [rows, HS2]
        o = np.zeros((a.shape[0], HS2), dtype=f)
        o[:, :HS] = a
        return o

    def pad_rz(a):          # [rows, 1002] -> [rows, 1004]
        o = np.zeros((a.shape[0], 2 * HS2), dtype=f)
        o[:, :HS] = a[:, :HS]
        o[:, HS2:HS2 + HS] = a[:, HS:]
        return o

    wrzh = pad_rz(np.concatenate([W_hh[:RZ].T, b_hh[None, :RZ]], axis=0))
    whn = padg(np.concatenate([W_hh[RZ:].T, b_hh[None, RZ:]], axis=0))
    wrzx = pad_rz(np.concatenate([W_ih[:RZ].T, b_ih[None, :RZ]], axis=0))
    wgvb = padg(bg[None, :] + Wg[:, HS:].T)          # [20, 502]
    wmvb = padg(np.ascontiguousarray(Wm[:, HS:].T))
    wgh = np.concatenate([padg(Wg[:, :HS].T), wgvb[0:1]], axis=0)  # [502, 502]
    wmh = np.concatenate([padg(Wm[:, :HS].T), wmvb[0:1]], axis=0)
    w12 = np.concatenate([np.concatenate([W1.T, W2.T], axis=1),
                          np.concatenate([b1, b2])[None, :]], axis=0).astype(f)
    ident = np.eye(128, dtype=f)

    ents, ncols = _pack_layout()
    import ml_dtypes
    bfdt = ml_dtypes.bfloat16

    def place(pack, name, arr):
        nr, c0, ncl = ents[name]
        assert arr.shape == (nr, ncl), (name, arr.shape, (nr, ncl))
        pack[0:nr, c0:c0 + ncl] = arr

    in_maps = []
    for c in range(NCORES):
        slc = slice(c * bs, (c + 1) * bs)
        Xc = X[slc]                                   # [bs, 20, 48]
        xt = Xc.transpose(2, 1, 0).reshape(XR, MAXN * bs)
        adjc = adj[slc]                               # [bs, 20, 20]

        pack = np.zeros((128, ncols), dtype=bfdt)
        place(pack, "pk", xt)
        for i, (o, s) in enumerate(CH):
            place(pack, f"wrzh{i}", wrzh[o:o + s])
            place(pack, f"whn{i}", whn[o:o + s])
            place(pack, f"wg{i}", wgh[o:o + s])
            place(pack, f"wm{i}", wmh[o:o + s])
            place(pack, f"w12{i}", w12[o:o + s])
        place(pack, "wrzx", wrzx)
        wfix = np.zeros((128, FIXCOLS), dtype=f)
        wfix[:, 0:128] = ident
        adjg = adjc.reshape(bs, MAXN * MAXN)
        wfix[:, 128:128 + MAXN * MAXN] = adjg[:128]
        wfix[:, 128 + MAXN * MAXN:] = adjg[128:]

        constt = np.zeros((MAXN * 2 * 128, HS2), dtype=f)
        try:
            import ml_dtypes
            bf = ml_dtypes.bfloat16
        except ImportError:
            bf = None
        gint = np.zeros((MAXN * 2 * 128, HS2),
                        dtype=(bf if bf is not None else f))
        cc = const[slc]                               # [bs, 20, 501]
        gc = ginb[slc]
        for v in range(MAXN):
            for t in range(2):
                r0 = (v * 2 + t) * 128
                constt[r0:r0 + 128, :HS] = cc[t * 128:(t + 1) * 128, v]
                constt[r0:r0 + 128, HS] = 1.0        # ones col -> bias rows
                gint[r0:r0 + 128, :HS] = gc[t * 128:(t + 1) * 128, v]
        in_maps.append(dict(wpack=pack, wfix=wfix, constt=constt, gint=gint,
                            wgvb=wgvb.astype(bfdt), wmvb=wmvb.astype(bfdt)))
    return in_maps


def _get_prog():
    global _PROG
    if _PROG is None:
        _PROG = _build_program()
    return _PROG


def kernel(**inputs):
    from concourse.bass_utils import run_bass_kernel_spmd
    nc = _get_prog()
    in_maps = _host_prep(**inputs)
    res = run_bass_kernel_spmd(nc, in_maps, core_ids=list(range(NCORES)))
    out = np.concatenate([r["out"] for r in res.results], axis=0)
    mu = np.ascontiguousarray(out[:, :NZ])
    logvar = np.ascontiguousarray(out[:, NZ:])
    return mu, logvar


# revision 23
# speedup vs baseline: 1.0221x; 1.0221x over previous
"""DVAE GNN message-passing kernel for 8 Trainium2 NeuronCores.

Data parallel over batch B=2048 -> 256 graphs/core (2 tiles of 128).
Each core runs the full 20-step topological scan, weights replicated.

Math (per sample b, step v in 0..19, Hfwd starts at 0):
  gated_u = sigmoid(Wg @ [H_u, vid_u] + bg) * (Wm @ [H_u, vid_u])
  Hin_v   = sum_{u<v} adj[b,u,v] * gated_u + const_v    (const_v is the
            u>=v part where H_u = 0 -- precomputed on host)
  H_v     = GRUCell(x_v, Hin_v)
  mu,lv   = W1 @ H_19 + b1, W2 @ H_19 + b2

Device schedule (per step): the adj-weighted message chains run on DVE as
fused scalar_tensor_tensor MACs with f32 accumulators seeded by the host
const tables, load-leveled across steps with an EDF prefill schedule
(future steps' chains accumulate early, during the current step's matmul
phases).  x-side r/z contributions + biases enter via a K=48 one-hot
matmul; the n-gate x-side (i_n + b_in, which the r-gate must NOT
multiply) comes from a host table added after the r*h_n product.  The
vertex-id one-hot enters via a per-step DMA update of the gated weights'
bias row.  PSUM: 8 banks = 2 transpose staging + 3 per batch tile
(r/z/n groups, reused by the gated zp/mp matmuls).
"""

import sys
import numpy as np

for _p in ("/opt/trn_rl_repo",):
    if _p not in sys.path:
        sys.path.insert(0, _p)

B, MAXN, NVT, HS, NZ = 2048, 20, 26, 501, 56
HS2 = HS + 1                  # 502: per-gate column pitch; col 501 = ones
NVT_EFF = NVT + MAXN          # 46
XDIM = NVT_EFF + 1            # 47
XR = XDIM + 1                 # 48: x rows incl ones row
NCORES = 8
BS = B // NCORES              # 256 samples per core
RZ = 2 * HS

# Haug^T chunks (501 hidden rows + ones row at 501)
CH = [(0, 128), (128, 128), (256, 128), (384, 118)]

CHAIN_CAP = 12   # prefill MAC budget per tile per step
LOOKAHEAD = 8    # chains w <= v + LOOKAHEAD may be prefilled at step v
AB_SLOTS = 8     # ring size for in-flight chain accumulators


def _chain_schedule():
    """EDF schedule for partial chain MACs (terms u <= w-2 of chain w).

    The final term u = w-1 runs as a fused stt right after G_{w-1} is
    produced at step w-1.  Returns sched[v] = [(w, u), ...] per step.
    """
    pend = [(w, u) for w in range(1, MAXN) for u in range(w - 1)]
    done = set()
    sched = [[] for _ in range(MAXN)]
    for v in range(MAXN):
        budget = CHAIN_CAP
        elig = [p for p in pend if p not in done and p[1] <= v - 1
                and p[0] - 1 >= v and p[0] <= v + LOOKAHEAD]
        elig.sort()
        for p in elig:
            mand = p[0] == v + 1
            if budget <= 0 and not mand:
                continue
            sched[v].append(p)
            done.add(p)
            budget -= 1
    assert len(done) == len(pend), (len(done), len(pend))
    for w in range(1, MAXN):
        for u in range(w - 1):
            vdone = next(v for v in range(MAXN) if (w, u) in sched[v])
            assert vdone <= w - 1
    return sched


def _pack_layout():
    """Column layout (fp32 elements) of the packed static tensor."""
    ents = {}
    col = 0

    def put(name, nrows, ncols):
        nonlocal col
        ents[name] = (nrows, col, ncols)
        col += ncols

    put("wrzx", XR, 2 * HS2)
    for i, (o, s) in enumerate(CH):
        put(f"wrzh{i}", s, 2 * HS2)
    for i, (o, s) in enumerate(CH):
        put(f"whn{i}", s, HS2)
    for i, (o, s) in enumerate(CH):
        put(f"wg{i}", s, HS2)
    for i, (o, s) in enumerate(CH):
        put(f"wm{i}", s, HS2)
    put("pk", XR, MAXN * BS)
    for i, (o, s) in enumerate(CH):
        put(f"w12{i}", s, 2 * NZ)
    return ents, col


FIXCOLS = 128 + 2 * MAXN * MAXN   # ident + adjg0 + adjg1 (f32)


_PROG = None


def _build_program():
    import concourse.tile as tile
    from concourse import bacc, mybir

    f32 = mybir.dt.float32
    mdt = mybir.dt.float32r
    bf16 = mybir.dt.bfloat16
    AF = mybir.ActivationFunctionType
    OP = mybir.AluOpType

    nc = bacc.Bacc("TRN2", target_bir_lowering=False, debug=False)

    ents, ncols = _pack_layout()
    d_wpack = nc.dram_tensor("wpack", [128, ncols], bf16,
                             kind="ExternalInput").ap()
    d_wfix = nc.dram_tensor("wfix", [128, FIXCOLS], f32,
                            kind="ExternalInput").ap()
    d_const = nc.dram_tensor("constt", [MAXN * 2 * 128, HS2], f32,
                             kind="ExternalInput").ap()
    d_gin = nc.dram_tensor("gint", [MAXN * 2 * 128, HS2], bf16,
                           kind="ExternalInput").ap()
    d_wgvb = nc.dram_tensor("wgvb", [MAXN, HS2], bf16,
                            kind="ExternalInput").ap()
    d_wmvb = nc.dram_tensor("wmvb", [MAXN, HS2], bf16,
                            kind="ExternalInput").ap()
    d_out = nc.dram_tensor("out", [BS, 2 * NZ], f32, kind="ExternalOutput").ap()

    sched = _chain_schedule()
    # first step at which chain w's accumulator opens (first partial MAC)
    first = {}
    for v in range(MAXN):
        for (w, u) in sched[v]:
            first.setdefault(w, v)
    first.setdefault(1, 0)   # chain 1 has no partials; cst read at GM(0)

    def mm(out, lhsT, rhs, start, stop):
        nc.tensor.matmul(out, lhsT, rhs, start=start, stop=stop)

    with tile.TileContext(nc) as tc:
        with (
            tc.tile_pool(name="statics", bufs=1) as sp,
            tc.tile_pool(name="gstore", bufs=2 * (MAXN - 1)) as gp,
            tc.tile_pool(name="csts", bufs=4) as cp,
            tc.tile_pool(name="gins", bufs=2) as gip,
            tc.tile_pool(name="hint", bufs=1) as hip,
            tc.tile_pool(name="ht", bufs=1) as htp,
            tc.tile_pool(name="work", bufs=1) as wp,
            tc.tile_pool(name="psum", bufs=1, space="PSUM") as pp,
        ):
            WPACK = sp.tile([128, ncols], bf16, tag="wpack", name="wpack")
            WFIX = sp.tile([128, FIXCOLS], f32, tag="wfix", name="wfix")
            nc.sync.dma_start(WFIX[:, :], d_wfix)
            NSPLIT = 12
            cuts = [ncols * i // NSPLIT for i in range(NSPLIT + 1)]
            for c0, c1 in zip(cuts[:-1], cuts[1:]):
                nc.sync.dma_start(WPACK[:, c0:c1], d_wpack[:, c0:c1])

            def sl(name, dt=None):
                nr, c0, ncl = ents[name]
                ap = WPACK[0:nr, c0:c0 + ncl]
                return ap.bitcast(dt) if dt else ap

            PK = sl("pk")
            WRZH = [sl(f"wrzh{i}") for i in range(4)]
            WHN = [sl(f"whn{i}") for i in range(4)]
            WRZX = sl("wrzx")
            WG = [sl(f"wg{i}") for i in range(4)]
            WM = [sl(f"wm{i}") for i in range(4)]
            W12 = [sl(f"w12{i}") for i in range(4)]
            IDN = WFIX[:, 0:128]
            ADJG = [WFIX[:, 128 + t * MAXN * MAXN:128 + (t + 1) * MAXN * MAXN]
                    for t in range(2)]

            # gated message vectors, one per (vertex, batch-tile)
            Gt = [[gp.tile([128, HS2], bf16, tag="g", name=f"g{u}_{t}")
                   for t in range(2)] for u in range(MAXN - 1)]

            cst, gin = {}, {}

            def dma_cst(w):
                for t in range(2):
                    c = cp.tile([128, HS2], f32, tag=f"cst{t}", name=f"cst{w}_{t}")
                    nc.sync.dma_start(c[:, :], d_const[(w * 2 + t) * 128:
                                                       (w * 2 + t + 1) * 128, :])
                    cst[(w, t)] = c

            def dma_gin(v):
                for t in range(2):
                    g = gip.tile([128, HS2], bf16, tag=f"gin{t}", name=f"gin{v}_{t}")
                    nc.sync.dma_start(g[:, :], d_gin[(v * 2 + t) * 128:
                                                     (v * 2 + t + 1) * 128, :])
                    gin[(v, t)] = g

            dma_cst(0)
            dma_cst(1)
            for w in sorted(first):
                if first[w] <= 1 and w > 1:
                    dma_cst(w)
            dma_gin(0)
            dma_gin(1)

            # chain accumulators: ab[(w, t)] is the running f32 partial sum
            # of chain w (const folded into the first MAC)
            ab = {}
            acc = {}           # (w, t) -> finished Hin tile
            for t in range(2):
                acc[(0, t)] = cst[(0, t)]

            def emit_macs(v, terms, offload=False):
                for idx, (w, u) in enumerate(terms):
                    for t in range(2):
                        off = t == 1
                        sc = ADJG[t][:, u * MAXN + w:u * MAXN + w + 1]
                        if (w, t) not in ab:
                            a = wp.tile([128, HS2], bf16, tag=f"ab{t}",
                                        bufs=AB_SLOTS, name=f"ab{w}_{t}")
                            ab[(w, t)] = a
                            srct = cst[(w, t)]
                        else:
                            a = ab[(w, t)]
                            srct = a
                        if off:
                            sm = wp.tile([128, HS2], bf16, tag=f"sm{t}",
                                         bufs=3, name=f"sm{w}_{u}_{t}")
                            nc.scalar.activation(sm[:, :], Gt[u][t][:, :],
                                                 AF.Copy, scale=sc)
                            nc.vector.tensor_tensor(a[:, :], srct[:, :],
                                                    sm[:, :], OP.add)
                        else:
                            nc.vector.scalar_tensor_tensor(
                                a[:, :], Gt[u][t][:, :], sc, srct[:, :],
                                OP.mult, OP.add)

            HINT = [None, None]
            HT = [None, None]
            htile = [None, None]
            psr = [None, None]
            psz = [None, None]
            psn = [None, None]

            def emit_tin(v, t):
                stage = pp.tile([128, 512], f32, tag="st", bufs=2,
                                name=f"sti{v}_{t}")
                a = acc[(v, t)]
                for i, (o, w) in enumerate(CH):
                    nc.tensor.transpose(
                        stage[0:w, 128 * i:128 * i + 128],
                        a[:, o:o + w], IDN[:, :])
                hi = hip.tile([128, 512], bf16, tag=f"hint{t}", name=f"hint{v}_{t}")
                nc.scalar.copy(hi[:, :], stage[:, :])
                HINT[t] = hi

            def emit_mm(v, t):
                xsl = PK[0:XR, v * BS + t * 128:v * BS + (t + 1) * 128]
                pr = pp.tile([128, 512], f32, tag=f"pa{t}", name=f"pr{v}_{t}")
                pz = pp.tile([128, 512], f32, tag=f"pb{t}", name=f"pz{v}_{t}")
                pn = pp.tile([128, 512], f32, tag=f"pc{t}", name=f"pn{v}_{t}")
                hi = HINT[t]
                mm(pr[:, 0:HS2], xsl, WRZX[:, 0:HS2], True, False)
                for i, (o, w) in enumerate(CH):
                    mm(pr[:, 0:HS2], hi[0:w, 128 * i:128 * i + 128],
                       WRZH[i][:, 0:HS2], False, i == 3)
                mm(pz[:, 0:HS2], xsl, WRZX[:, HS2:2 * HS2], True, False)
                for i, (o, w) in enumerate(CH):
                    mm(pz[:, 0:HS2], hi[0:w, 128 * i:128 * i + 128],
                       WRZH[i][:, HS2:2 * HS2], False, i == 3)
                for i, (o, w) in enumerate(CH):
                    mm(pn[:, 0:HS2], hi[0:w, 128 * i:128 * i + 128],
                       WHN[i][:, 0:HS2], i == 0, i == 3)
                psr[t], psz[t], psn[t] = pr, pz, pn

            def emit_act(v, t):
                r = wp.tile([128, HS2], bf16, tag=f"r{t}", name=f"r{v}_{t}")
                z = wp.tile([128, HS2], bf16, tag=f"z{t}", name=f"z{v}_{t}")
                nc.scalar.activation(r[:, :], psr[t][:, 0:HS2], AF.Sigmoid)
                nc.scalar.activation(z[:, :], psz[t][:, 0:HS2], AF.Sigmoid)
                tmp = wp.tile([128, HS], f32, tag=f"tmp{t}", name=f"tmp{v}_{t}")
                # n = tanh(r * h_n + (i_n + b_in));  h_n (incl b_hn) in PSUM
                nc.vector.tensor_tensor(tmp[:, :], r[:, 0:HS], psn[t][:, 0:HS],
                                        OP.mult)
                nc.gpsimd.tensor_tensor(tmp[:, :], tmp[:, :],
                                        gin[(v, t)][:, 0:HS], OP.add)
                n = wp.tile([128, HS], bf16, tag=f"n{t}", name=f"n{v}_{t}")
                nc.scalar.activation(n[:, :], tmp[:, :], AF.Tanh)
                a = acc[(v, t)]
                d = wp.tile([128, HS], f32, tag=f"d{t}", name=f"d{v}_{t}")
                nc.gpsimd.tensor_tensor(d[:, :], a[:, 0:HS], n[:, :], OP.subtract)
                h = wp.tile([128, HS2], f32, tag=f"h{t}", bufs=2,
                            name=f"h{v}_{t}")
                nc.gpsimd.tensor_tensor(h[:, 0:HS], d[:, :], z[:, 0:HS], OP.mult)
                nc.vector.tensor_tensor(h[:, 0:HS], h[:, 0:HS], n[:, :], OP.add)
                if v < 2:
                    nc.gpsimd.memset(h[:, HS:HS2], 1.0)  # ones col -> bias row
                htile[t] = h

            def emit_th(v, t):
                stage = pp.tile([128, 512], f32, tag="st", bufs=2,
                                name=f"sth{v}_{t}")
                h = htile[t]
                for i, (o, w) in enumerate(CH):
                    nc.tensor.transpose(
                        stage[0:w, 128 * i:128 * i + 128],
                        h[:, o:o + w], IDN[:, :])
                ht = htp.tile([128, 512], bf16, tag=f"ht{t}", name=f"ht{v}_{t}")
                nc.scalar.copy(ht[:, :], stage[:, :])
                HT[t] = ht

            def emit_gm(v, t):
                zp = pp.tile([128, 512], f32, tag=f"pa{t}", name=f"zp{v}_{t}")
                mp = pp.tile([128, 512], f32, tag=f"pb{t}", name=f"mp{v}_{t}")
                ht = HT[t]
                for i, (o, w) in enumerate(CH):
                    mm(zp[:, 0:HS2], ht[0:w, 128 * i:128 * i + 128],
                       WG[i][:, 0:HS2], i == 0, i == 3)
                for i, (o, w) in enumerate(CH):
                    mm(mp[:, 0:HS2], ht[0:w, 128 * i:128 * i + 128],
                       WM[i][:, 0:HS2], i == 0, i == 3)
                sg = wp.tile([128, HS2], bf16, tag=f"sg{t}", name=f"sg{v}_{t}")
                nc.scalar.activation(sg[:, :], zp[:, 0:HS2], AF.Sigmoid)
                g = Gt[v][t]
                nc.vector.tensor_tensor(g[:, :], sg[:, :], mp[:, 0:HS2], OP.mult)
                # finish chain v+1: acc = a_{v,v+1} * G_v + partial
                w1 = v + 1
                sc = ADJG[t][:, v * MAXN + w1:v * MAXN + w1 + 1]
                src = ab.get((w1, t), cst.get((w1, t)))
                at = wp.tile([128, HS2], f32, tag=f"acc{t}", bufs=2,
                             name=f"acc{w1}_{t}")
                nc.vector.scalar_tensor_tensor(at[:, :], g[:, :], sc,
                                               src[:, :], OP.mult, OP.add)
                acc[(w1, t)] = at

            for v in range(MAXN):
                fin = [p for p in sched[v] if p[0] == v + 1]
                pre = [p for p in sched[v] if p[0] != v + 1]
                emit_macs(v, fin)

                emit_tin(v, 0)
                emit_tin(v, 1)
                emit_mm(v, 0)
                emit_mm(v, 1)
                emit_act(v, 0)
                emit_th(v, 0)
                emit_act(v, 1)
                if v < MAXN - 1:
                    emit_gm(v, 0)
                    emit_th(v, 1)
                    emit_gm(v, 1)
                    emit_macs(v, pre)
                else:
                    emit_th(v, 1)
                if v < MAXN - 1:
                    # gated bias+vid row update for step v+1
                    nc.sync.dma_start(WG[3][117:118, 0:HS2],
                                      d_wgvb[v + 1:v + 2, :])
                    nc.sync.dma_start(WM[3][117:118, 0:HS2],
                                      d_wmvb[v + 1:v + 2, :])
                # prefetch streams for upcoming steps
                for w in sorted(first):
                    if first[w] == v + 1 and (w, 0) not in cst:
                        dma_cst(w)
                if v + 2 < MAXN:
                    dma_gin(v + 2)

            # readout from HT (H_19)
            for t in range(2):
                op = pp.tile([128, 512], f32, tag=f"pc{t}", name=f"op{t}")
                ht = HT[t]
                for i, (o, w) in enumerate(CH):
                    mm(op[:, 0:2 * NZ], ht[0:w, 128 * i:128 * i + 128],
                       W12[i][:, :], i == 0, i == 3)
                ob = wp.tile([128, 2 * NZ], f32, tag=f"ob{t}", name=f"ob{t}")
                nc.scalar.copy(ob[:, :], op[:, 0:2 * NZ])
                nc.sync.dma_start(d_out[t * 128:(t + 1) * 128, :], ob[:, :])

    nc.compile()
    return nc


def _host_prep(types, feats, adj, Wg, bg, Wm, W_ih, b_ih, W_hh, b_hh, W1, b1,
               W2, b2):
    f = np.float32
    types = np.asarray(types).astype(np.int64)
    feats = np.asarray(feats, dtype=f)
    adj = np.asarray(adj, dtype=f)
    Wg, bg, Wm = np.asarray(Wg, f), np.asarray(bg, f), np.asarray(Wm, f)
    W_ih, b_ih = np.asarray(W_ih, f), np.asarray(b_ih, f)
    W_hh, b_hh = np.asarray(W_hh, f), np.asarray(b_hh, f)
    W1, b1 = np.asarray(W1, f), np.asarray(b1, f)
    W2, b2 = np.asarray(W2, f), np.asarray(b2, f)

    bsz = types.shape[0]
    bs = bsz // NCORES

    # X^T with ones row: [48, MAXN*bs] slices per core
    X = np.zeros((bsz, MAXN, XR), dtype=f)
    onehot = np.eye(NVT_EFF, dtype=f)[types.reshape(-1) % NVT_EFF]
    X[:, :, :NVT_EFF] = onehot.reshape(bsz, MAXN, NVT_EFF)
    X[:, :, NVT_EFF] = feats
    X[:, :, XDIM] = 1.0

    # constant gated vectors c_u for zero hidden state
    zg = 1.0 / (1.0 + np.exp(-(bg[None, :] + Wg[:, HS:].T)))   # [20, 501]
    C = (zg * Wm[:, HS:].T).astype(f)
    umask = (np.arange(MAXN)[:, None] >= np.arange(MAXN)[None, :]).astype(f)
    const = np.einsum('buv,uh->bvh', adj * umask[None, :, :], C).astype(f)

    # i_n + b_in per (b, v): one-hot gather instead of a matmul
    Wn = W_ih[RZ:]                                   # [501, 47]
    ginb = Wn.T[types.reshape(-1) % NVT_EFF]         # [B*20, 501] type rows
    ginb = ginb.reshape(bsz, MAXN, HS) + feats[..., None] * Wn[:, NVT_EFF]
    ginb = (ginb + b_ih[RZ:]).astype(f)              # [B, 20, 501]

    def padg(a):            # [rows, HS] -> [rows, HS2]
        o = np.zeros((a.shape[0], HS2), dtype=f)
        o[:, :HS] = a
        return o

    def pad_rz(a):          # [rows, 1002] -> [rows, 1004]
        o = np.zeros((a.shape[0], 2 * HS2), dtype=f)
        o[:, :HS] = a[:, :HS]
        o[:, HS2:HS2 + HS] = a[:, HS:]
        return o

    wrzh = pad_rz(np.concatenate([W_hh[:RZ].T, b_hh[None, :RZ]], axis=0))
    whn = padg(np.concatenate([W_hh[RZ:].T, b_hh[None, RZ:]], axis=0))
    wrzx = pad_rz(np.concatenate([W_ih[:RZ].T, b_ih[None, :RZ]], axis=0))
    wgvb = padg(bg[None, :] + Wg[:, HS:].T)          # [20, 502]
    wmvb = padg(np.ascontiguousarray(Wm[:, HS:].T))
    wgh = np.concatenate([padg(Wg[:, :HS].T), wgvb[0:1]], axis=0)  # [502, 502]
    wmh = np.concatenate([padg(Wm[:, :HS].T), wmvb[0:1]], axis=0)
    w12 = np.concatenate([np.concatenate([W1.T, W2.T], axis=1),
                          np.concatenate([b1, b2])[None, :]], axis=0).astype(f)
    ident = np.eye(128, dtype=f)

    ents, ncols = _pack_layout()
    import ml_dtypes
    bfdt = ml_dtypes.bfloat16

    def place(pack, name, arr):
        nr, c0, ncl = ents[name]
        assert arr.shape == (nr, ncl), (name, arr.shape, (nr, ncl))
        pack[0:nr, c0:c0 + ncl] = arr

    in_maps = []
    for c in range(NCORES):
        slc = slice(c * bs, (c + 1) * bs)
        Xc = X[slc]                                   # [bs, 20, 48]
        xt = Xc.transpose(2, 1, 0).reshape(XR, MAXN * bs)
        adjc = adj[slc]                               # [bs, 20, 20]

        pack = np.zeros((128, ncols), dtype=bfdt)
        place(pack, "pk", xt)
        for i, (o, s) in enumerate(CH):
            place(pack, f"wrzh{i}", wrzh[o:o + s])
            place(pack, f"whn{i}", whn[o:o + s])
            place(pack, f"wg{i}", wgh[o:o + s])
            place(pack, f"wm{i}", wmh[o:o + s])
            place(pack, f"w12{i}", w12[o:o + s])
        place(pack, "wrzx", wrzx)
        wfix = np.zeros((128, FIXCOLS), dtype=f)
        wfix[:, 0:128] = ident
        adjg = adjc.reshape(bs, MAXN * MAXN)
        wfix[:, 128:128 + MAXN * MAXN] = adjg[:128]
        wfix[:, 128 + MAXN * MAXN:] = adjg[128:]

        constt = np.zeros((MAXN * 2 * 128, HS2), dtype=f)
        try:
            import ml_dtypes
            bf = ml_dtypes.bfloat16
        except ImportError:
            bf = None
        gint = np.zeros((MAXN * 2 * 128, HS2),
                        dtype=(bf if bf is not None else f))
        cc = const[slc]                               # [bs, 20, 501]
        gc = ginb[slc]
        for v in range(MAXN):
            for t in range(2):
                r0 = (v * 2 + t) * 128
                constt[r0:r0 + 128, :HS] = cc[t * 128:(t + 1) * 128, v]
                constt[r0:r0 + 128, HS] = 1.0        # ones col -> bias rows
                gint[r0:r0 + 128, :HS] = gc[t * 128:(t + 1) * 128, v]
        in_maps.append(dict(wpack=pack, wfix=wfix, constt=constt, gint=gint,
                            wgvb=wgvb.astype(bfdt), wmvb=wmvb.astype(bfdt)))
    return in_maps


def _get_prog():
    global _PROG
    if _PROG is None:
        _PROG = _build_program()
    return _PROG


def kernel(**inputs):
    from concourse.bass_utils import run_bass_kernel_spmd
    nc = _get_prog()
    in_maps = _host_prep(**inputs)
    res = run_bass_kernel_spmd(nc, in_maps, core_ids=list(range(NCORES)))
    out = np.concatenate([r["out"] for r in res.results], axis=0)
    mu = np.ascontiguousarray(out[:, :NZ])
    logvar = np.ascontiguousarray(out[:, NZ:])
    return mu, logvar


# revision 25
# speedup vs baseline: 1.0493x; 1.0267x over previous
"""DVAE GNN message-passing kernel for 8 Trainium2 NeuronCores.

Data parallel over batch B=2048 -> 256 graphs/core (2 tiles of 128).
Each core runs the full 20-step topological scan, weights replicated.

Math (per sample b, step v in 0..19, Hfwd starts at 0):
  gated_u = sigmoid(Wg @ [H_u, vid_u] + bg) * (Wm @ [H_u, vid_u])
  Hin_v   = sum_{u<v} adj[b,u,v] * gated_u + const_v    (const_v is the
            u>=v part where H_u = 0 -- precomputed on host)
  H_v     = GRUCell(x_v, Hin_v)
  mu,lv   = W1 @ H_19 + b1, W2 @ H_19 + b2

Device schedule (per step): the adj-weighted message chains run on DVE as
fused scalar_tensor_tensor MACs with f32 accumulators seeded by the host
const tables, load-leveled across steps with an EDF prefill schedule
(future steps' chains accumulate early, during the current step's matmul
phases).  x-side r/z contributions + biases enter via a K=48 one-hot
matmul; the n-gate x-side (i_n + b_in, which the r-gate must NOT
multiply) comes from a host table added after the r*h_n product.  The
vertex-id one-hot enters via a per-step DMA update of the gated weights'
bias row.  PSUM: 8 banks = 2 transpose staging + 3 per batch tile
(r/z/n groups, reused by the gated zp/mp matmuls).
"""

import sys
import numpy as np

for _p in ("/opt/trn_rl_repo",):
    if _p not in sys.path:
        sys.path.insert(0, _p)

B, MAXN, NVT, HS, NZ = 2048, 20, 26, 501, 56
HS2 = HS + 1                  # 502: per-gate column pitch; col 501 = ones
NVT_EFF = NVT + MAXN          # 46
XDIM = NVT_EFF + 1            # 47
XR = XDIM + 1                 # 48: x rows incl ones row
NCORES = 8
BS = B // NCORES              # 256 samples per core
RZ = 2 * HS

# Haug^T chunks (501 hidden rows + ones row at 501)
CH = [(0, 128), (128, 128), (256, 128), (384, 118)]

CHAIN_CAP = 10   # prefill MAC budget per tile per step
LOOKAHEAD = 8    # chains w <= v + LOOKAHEAD may be prefilled at step v
AB_SLOTS = 10    # ring size for in-flight chain accumulators


def _chain_schedule():
    """EDF schedule for partial chain MACs (terms u <= w-2 of chain w).

    The final term u = w-1 runs as a fused stt right after G_{w-1} is
    produced at step w-1.  Returns sched[v] = [(w, u), ...] per step.
    """
    pend = [(w, u) for w in range(1, MAXN) for u in range(w - 1)]
    done = set()
    sched = [[] for _ in range(MAXN)]
    for v in range(MAXN):
        budget = CHAIN_CAP
        elig = [p for p in pend if p not in done and p[1] <= v - 1
                and p[0] - 1 >= v and p[0] <= v + LOOKAHEAD]
        elig.sort()
        for p in elig:
            mand = p[0] == v + 1
            if budget <= 0 and not mand:
                continue
            sched[v].append(p)
            done.add(p)
            budget -= 1
    assert len(done) == len(pend), (len(done), len(pend))
    for w in range(1, MAXN):
        for u in range(w - 1):
            vdone = next(v for v in range(MAXN) if (w, u) in sched[v])
            assert vdone <= w - 1
    return sched


def _pack_layout():
    """Column layout (fp32 elements) of the packed static tensor."""
    ents = {}
    col = 0

    def put(name, nrows, ncols):
        nonlocal col
        ents[name] = (nrows, col, ncols)
        col += ncols

    put("wrzx", XR, 2 * HS2)
    for i, (o, s) in enumerate(CH):
        put(f"wrzh{i}", s, 2 * HS2)
    for i, (o, s) in enumerate(CH):
        put(f"whn{i}", s, HS2)
    for i, (o, s) in enumerate(CH):
        put(f"wg{i}", s, HS2)
    for i, (o, s) in enumerate(CH):
        put(f"wm{i}", s, HS2)
    put("pk", XR, MAXN * BS)
    for i, (o, s) in enumerate(CH):
        put(f"w12{i}", s, 2 * NZ)
    return ents, col


FIXCOLS = 128 + 2 * MAXN * MAXN   # ident + adjg0 + adjg1 (f32)


_PROG = None


def _build_program():
    import concourse.tile as tile
    from concourse import bacc, mybir

    f32 = mybir.dt.float32
    mdt = mybir.dt.float32r
    bf16 = mybir.dt.bfloat16
    AF = mybir.ActivationFunctionType
    OP = mybir.AluOpType

    nc = bacc.Bacc("TRN2", target_bir_lowering=False, debug=False)

    ents, ncols = _pack_layout()
    d_wpack = nc.dram_tensor("wpack", [128, ncols], bf16,
                             kind="ExternalInput").ap()
    d_wfix = nc.dram_tensor("wfix", [128, FIXCOLS], f32,
                            kind="ExternalInput").ap()
    d_const = nc.dram_tensor("constt", [MAXN * 2 * 128, HS2], f32,
                             kind="ExternalInput").ap()
    d_gin = nc.dram_tensor("gint", [MAXN * 2 * 128, HS2], bf16,
                           kind="ExternalInput").ap()
    d_wgvb = nc.dram_tensor("wgvb", [MAXN, HS2], bf16,
                            kind="ExternalInput").ap()
    d_wmvb = nc.dram_tensor("wmvb", [MAXN, HS2], bf16,
                            kind="ExternalInput").ap()
    d_out = nc.dram_tensor("out", [BS, 2 * NZ], f32, kind="ExternalOutput").ap()

    sched = _chain_schedule()
    # first step at which chain w's accumulator opens (first partial MAC)
    first = {}
    for v in range(MAXN):
        for (w, u) in sched[v]:
            first.setdefault(w, v)
    first.setdefault(1, 0)   # chain 1 has no partials; cst read at GM(0)

    def mm(out, lhsT, rhs, start, stop):
        nc.tensor.matmul(out, lhsT, rhs, start=start, stop=stop)

    with tile.TileContext(nc) as tc:
        with (
            tc.tile_pool(name="statics", bufs=1) as sp,
            tc.tile_pool(name="gstore", bufs=2 * (MAXN - 1)) as gp,
            tc.tile_pool(name="csts", bufs=4) as cp,
            tc.tile_pool(name="gins", bufs=2) as gip,
            tc.tile_pool(name="hint", bufs=1) as hip,
            tc.tile_pool(name="ht", bufs=1) as htp,
            tc.tile_pool(name="work", bufs=1) as wp,
            tc.tile_pool(name="psum", bufs=1, space="PSUM") as pp,
        ):
            WPACK = sp.tile([128, ncols], bf16, tag="wpack", name="wpack")
            WFIX = sp.tile([128, FIXCOLS], f32, tag="wfix", name="wfix")
            nc.sync.dma_start(WFIX[:, :], d_wfix)
            NSPLIT = 12
            cuts = [ncols * i // NSPLIT for i in range(NSPLIT + 1)]
            for c0, c1 in zip(cuts[:-1], cuts[1:]):
                nc.sync.dma_start(WPACK[:, c0:c1], d_wpack[:, c0:c1])

            def sl(name, dt=None):
                nr, c0, ncl = ents[name]
                ap = WPACK[0:nr, c0:c0 + ncl]
                return ap.bitcast(dt) if dt else ap

            PK = sl("pk")
            WRZH = [sl(f"wrzh{i}") for i in range(4)]
            WHN = [sl(f"whn{i}") for i in range(4)]
            WRZX = sl("wrzx")
            WG = [sl(f"wg{i}") for i in range(4)]
            WM = [sl(f"wm{i}") for i in range(4)]
            W12 = [sl(f"w12{i}") for i in range(4)]
            IDN = WFIX[:, 0:128]
            ADJG = [WFIX[:, 128 + t * MAXN * MAXN:128 + (t + 1) * MAXN * MAXN]
                    for t in range(2)]

            # gated message vectors, one per (vertex, batch-tile)
            Gt = [[gp.tile([128, HS2], bf16, tag="g", name=f"g{u}_{t}")
                   for t in range(2)] for u in range(MAXN - 1)]

            cst, gin = {}, {}

            def dma_cst(w):
                for t in range(2):
                    c = cp.tile([128, HS2], f32, tag=f"cst{t}", name=f"cst{w}_{t}")
                    nc.sync.dma_start(c[:, :], d_const[(w * 2 + t) * 128:
                                                       (w * 2 + t + 1) * 128, :])
                    cst[(w, t)] = c

            def dma_gin(v):
                for t in range(2):
                    g = gip.tile([128, HS2], bf16, tag=f"gin{t}", name=f"gin{v}_{t}")
                    nc.sync.dma_start(g[:, :], d_gin[(v * 2 + t) * 128:
                                                     (v * 2 + t + 1) * 128, :])
                    gin[(v, t)] = g

            dma_cst(0)
            dma_cst(1)
            for w in sorted(first):
                if first[w] <= 1 and w > 1:
                    dma_cst(w)
            dma_gin(0)
            dma_gin(1)

            # chain accumulators: ab[(w, t)] is the running f32 partial sum
            # of chain w (const folded into the first MAC)
            ab = {}
            acc = {}           # (w, t) -> finished Hin tile
            for t in range(2):
                acc[(0, t)] = cst[(0, t)]

            def emit_macs(v, terms, offload=False):
                for idx, (w, u) in enumerate(terms):
                    for t in range(2):
                        off = t == 1
                        sc = ADJG[t][:, u * MAXN + w:u * MAXN + w + 1]
                        if (w, t) not in ab:
                            a = wp.tile([128, HS2], bf16, tag=f"ab{t}",
                                        bufs=AB_SLOTS, name=f"ab{w}_{t}")
                            ab[(w, t)] = a
                            srct = cst[(w, t)]
                        else:
                            a = ab[(w, t)]
                            srct = a
                        if off:
                            sm = wp.tile([128, HS2], bf16, tag=f"sm{t}",
                                         bufs=3, name=f"sm{w}_{u}_{t}")
                            nc.scalar.activation(sm[:, :], Gt[u][t][:, :],
                                                 AF.Copy, scale=sc)
                            nc.vector.tensor_tensor(a[:, :], srct[:, :],
                                                    sm[:, :], OP.add)
                        else:
                            nc.vector.scalar_tensor_tensor(
                                a[:, :], Gt[u][t][:, :], sc, srct[:, :],
                                OP.mult, OP.add)

            HINT = [None, None]
            HT = [None, None]
            htile = [None, None]
            psr = [None, None]
            psz = [None, None]
            psn = [None, None]

            def emit_tin(v, t):
                stage = pp.tile([128, 512], f32, tag="st", bufs=2,
                                name=f"sti{v}_{t}")
                a = acc[(v, t)]
                for i, (o, w) in enumerate(CH):
                    nc.tensor.transpose(
                        stage[0:w, 128 * i:128 * i + 128],
                        a[:, o:o + w], IDN[:, :])
                hi = hip.tile([128, 512], bf16, tag=f"hint{t}", name=f"hint{v}_{t}")
                nc.scalar.copy(hi[:, :], stage[:, :])
                HINT[t] = hi

            def emit_mm(v, t):
                xsl = PK[0:XR, v * BS + t * 128:v * BS + (t + 1) * 128]
                pr = pp.tile([128, 512], f32, tag=f"pa{t}", name=f"pr{v}_{t}")
                pz = pp.tile([128, 512], f32, tag=f"pb{t}", name=f"pz{v}_{t}")
                pn = pp.tile([128, 512], f32, tag=f"pc{t}", name=f"pn{v}_{t}")
                hi = HINT[t]
                mm(pr[:, 0:HS2], xsl, WRZX[:, 0:HS2], True, False)
                for i, (o, w) in enumerate(CH):
                    mm(pr[:, 0:HS2], hi[0:w, 128 * i:128 * i + 128],
                       WRZH[i][:, 0:HS2], False, i == 3)
                mm(pz[:, 0:HS2], xsl, WRZX[:, HS2:2 * HS2], True, False)
                for i, (o, w) in enumerate(CH):
                    mm(pz[:, 0:HS2], hi[0:w, 128 * i:128 * i + 128],
                       WRZH[i][:, HS2:2 * HS2], False, i == 3)
                for i, (o, w) in enumerate(CH):
                    mm(pn[:, 0:HS2], hi[0:w, 128 * i:128 * i + 128],
                       WHN[i][:, 0:HS2], i == 0, i == 3)
                psr[t], psz[t], psn[t] = pr, pz, pn

            def emit_act(v, t):
                r = wp.tile([128, HS2], bf16, tag=f"r{t}", name=f"r{v}_{t}")
                z = wp.tile([128, HS2], bf16, tag=f"z{t}", name=f"z{v}_{t}")
                nc.scalar.activation(r[:, :], psr[t][:, 0:HS2], AF.Sigmoid)
                nc.scalar.activation(z[:, :], psz[t][:, 0:HS2], AF.Sigmoid)
                tmp = wp.tile([128, HS], f32, tag=f"tmp{t}", name=f"tmp{v}_{t}")
                # n = tanh(r * h_n + (i_n + b_in));  h_n (incl b_hn) in PSUM
                nc.vector.tensor_tensor(tmp[:, :], r[:, 0:HS], psn[t][:, 0:HS],
                                        OP.mult)
                nc.gpsimd.tensor_tensor(tmp[:, :], tmp[:, :],
                                        gin[(v, t)][:, 0:HS], OP.add)
                n = wp.tile([128, HS], bf16, tag=f"n{t}", name=f"n{v}_{t}")
                nc.scalar.activation(n[:, :], tmp[:, :], AF.Tanh)
                a = acc[(v, t)]
                d = wp.tile([128, HS], f32, tag=f"d{t}", name=f"d{v}_{t}")
                nc.gpsimd.tensor_tensor(d[:, :], a[:, 0:HS], n[:, :], OP.subtract)
                h = wp.tile([128, HS2], f32, tag=f"h{t}", bufs=2,
                            name=f"h{v}_{t}")
                nc.gpsimd.tensor_tensor(h[:, 0:HS], d[:, :], z[:, 0:HS], OP.mult)
                nc.vector.tensor_tensor(h[:, 0:HS], h[:, 0:HS], n[:, :], OP.add)
                if v < 2:
                    nc.gpsimd.memset(h[:, HS:HS2], 1.0)  # ones col -> bias row
                htile[t] = h

            def emit_th(v, t):
                stage = pp.tile([128, 512], f32, tag="st", bufs=2,
                                name=f"sth{v}_{t}")
                h = htile[t]
                for i, (o, w) in enumerate(CH):
                    nc.tensor.transpose(
                        stage[0:w, 128 * i:128 * i + 128],
                        h[:, o:o + w], IDN[:, :])
                ht = htp.tile([128, 512], bf16, tag=f"ht{t}", name=f"ht{v}_{t}")
                nc.scalar.copy(ht[:, :], stage[:, :])
                HT[t] = ht

            def emit_gm(v, t):
                zp = pp.tile([128, 512], f32, tag=f"pa{t}", name=f"zp{v}_{t}")
                mp = pp.tile([128, 512], f32, tag=f"pb{t}", name=f"mp{v}_{t}")
                ht = HT[t]
                for i, (o, w) in enumerate(CH):
                    mm(zp[:, 0:HS2], ht[0:w, 128 * i:128 * i + 128],
                       WG[i][:, 0:HS2], i == 0, i == 3)
                for i, (o, w) in enumerate(CH):
                    mm(mp[:, 0:HS2], ht[0:w, 128 * i:128 * i + 128],
                       WM[i][:, 0:HS2], i == 0, i == 3)
                sg = wp.tile([128, HS2], bf16, tag=f"sg{t}", name=f"sg{v}_{t}")
                nc.scalar.activation(sg[:, :], zp[:, 0:HS2], AF.Sigmoid)
                g = Gt[v][t]
                nc.vector.tensor_tensor(g[:, :], sg[:, :], mp[:, 0:HS2], OP.mult)
                # finish chain v+1: acc = a_{v,v+1} * G_v + partial
                w1 = v + 1
                sc = ADJG[t][:, v * MAXN + w1:v * MAXN + w1 + 1]
                src = ab.get((w1, t), cst.get((w1, t)))
                at = wp.tile([128, HS2], f32, tag=f"acc{t}", bufs=2,
                             name=f"acc{w1}_{t}")
                nc.vector.scalar_tensor_tensor(at[:, :], g[:, :], sc,
                                               src[:, :], OP.mult, OP.add)
                acc[(w1, t)] = at

            for v in range(MAXN):
                fin = [p for p in sched[v] if p[0] == v + 1]
                pre = [p for p in sched[v] if p[0] != v + 1]
                emit_macs(v, fin)

                emit_tin(v, 0)
                emit_tin(v, 1)
                emit_mm(v, 0)
                emit_mm(v, 1)
                emit_act(v, 0)
                emit_th(v, 0)
                emit_act(v, 1)
                if v < MAXN - 1:
                    emit_gm(v, 0)
                    emit_th(v, 1)
                    emit_gm(v, 1)
                    emit_macs(v, pre)
                else:
                    emit_th(v, 1)
                if v < MAXN - 1:
                    # gated bias+vid row update for step v+1
                    nc.sync.dma_start(WG[3][117:118, 0:HS2],
                                      d_wgvb[v + 1:v + 2, :])
                    nc.sync.dma_start(WM[3][117:118, 0:HS2],
                                      d_wmvb[v + 1:v + 2, :])
                # prefetch streams for upcoming steps
                for w in sorted(first):
                    if first[w] == v + 1 and (w, 0) not in cst:
                        dma_cst(w)
                if v + 2 < MAXN:
                    dma_gin(v + 2)

            # readout from HT (H_19)
            for t in range(2):
                op = pp.tile([128, 512], f32, tag=f"pc{t}", name=f"op{t}")
                ht = HT[t]
                for i, (o, w) in enumerate(CH):
                    mm(op[:, 0:2 * NZ], ht[0:w, 128 * i:128 * i + 128],
                       W12[i][:, :], i == 0, i == 3)
                ob = wp.tile([128, 2 * NZ], f32, tag=f"ob{t}", name=f"ob{t}")
                nc.scalar.copy(ob[:, :], op[:, 0:2 * NZ])
                nc.sync.dma_start(d_out[t * 128:(t + 1) * 128, :], ob[:, :])

    nc.compile()
    return nc


def _host_prep(types, feats, adj, Wg, bg, Wm, W_ih, b_ih, W_hh, b_hh, W1, b1,
               W2, b2):
    f = np.float32
    types = np.asarray(types).astype(np.int64)
    feats = np.asarray(feats, dtype=f)
    adj = np.asarray(adj, dtype=f)
    Wg, bg, Wm = np.asarray(Wg, f), np.asarray(bg, f), np.asarray(Wm, f)
    W_ih, b_ih = np.asarray(W_ih, f), np.asarray(b_ih, f)
    W_hh, b_hh = np.asarray(W_hh, f), np.asarray(b_hh, f)
    W1, b1 = np.asarray(W1, f), np.asarray(b1, f)
    W2, b2 = np.asarray(W2, f), np.asarray(b2, f)

    bsz = types.shape[0]
    bs = bsz // NCORES

    # X^T with ones row: [48, MAXN*bs] slices per core
    X = np.zeros((bsz, MAXN, XR), dtype=f)
    onehot = np.eye(NVT_EFF, dtype=f)[types.reshape(-1) % NVT_EFF]
    X[:, :, :NVT_EFF] = onehot.reshape(bsz, MAXN, NVT_EFF)
    X[:, :, NVT_EFF] = feats
    X[:, :, XDIM] = 1.0

    # constant gated vectors c_u for zero hidden state
    zg = 1.0 / (1.0 + np.exp(-(bg[None, :] + Wg[:, HS:].T)))   # [20, 501]
    C = (zg * Wm[:, HS:].T).astype(f)
    umask = (np.arange(MAXN)[:, None] >= np.arange(MAXN)[None, :]).astype(f)
    const = np.einsum('buv,uh->bvh', adj * umask[None, :, :], C).astype(f)

    # i_n + b_in per (b, v): one-hot gather instead of a matmul
    Wn = W_ih[RZ:]                                   # [501, 47]
    ginb = Wn.T[types.reshape(-1) % NVT_EFF]         # [B*20, 501] type rows
    ginb = ginb.reshape(bsz, MAXN, HS) + feats[..., None] * Wn[:, NVT_EFF]
    ginb = (ginb + b_ih[RZ:]).astype(f)              # [B, 20, 501]

    def padg(a):            # [rows, HS] -> [rows, HS2]
        o = np.zeros((a.shape[0], HS2), dtype=f)
        o[:, :HS] = a
        return o

    def pad_rz(a):          # [rows, 1002] -> [rows, 1004]
        o = np.zeros((a.shape[0], 2 * HS2), dtype=f)
        o[:, :HS] = a[:, :HS]
        o[:, HS2:HS2 + HS] = a[:, HS:]
        return o

    wrzh = pad_rz(np.concatenate([W_hh[:RZ].T, b_hh[None, :RZ]], axis=0))
    whn = padg(np.concatenate([W_hh[RZ:].T, b_hh[None, RZ:]], axis=0))
    wrzx = pad_rz(np.concatenate([W_ih[:RZ].T, b_ih[None, :RZ]], axis=0))
    wgvb = padg(bg[None, :] + Wg[:, HS:].T)          # [20, 502]
    wmvb = padg(np.ascontiguousarray(Wm[:, HS:].T))
    wgh = np.concatenate([padg(Wg[:, :HS].T), wgvb[0:1]], axis=0)  # [502, 502]
    wmh = np.concatenate([padg(Wm[:, :HS].T), wmvb[0:1]], axis=0)
    w12 = np.concatenate([np.concatenate([W1.T, W2.T], axis=1),
                          np.concatenate([b1, b2])[None, :]], axis=0).astype(f)
    ident = np.eye(128, dtype=f)

    ents, ncols = _pack_layout()
    import ml_dtypes
    bfdt = ml_dtypes.bfloat16

    def place(pack, name, arr):
        nr, c0, ncl = ents[name]
        assert arr.shape == (nr, ncl), (name, arr.shape, (nr, ncl))
        pack[0:nr, c0:c0 + ncl] = arr

    in_maps = []
    for c in range(NCORES):
        slc = slice(c * bs, (c + 1) * bs)
        Xc = X[slc]                                   # [bs, 20, 48]
        xt = Xc.transpose(2, 1, 0).reshape(XR, MAXN * bs)
        adjc = adj[slc]                               # [bs, 20, 20]

        pack = np.zeros((128, ncols), dtype=bfdt)
        place(pack, "pk", xt)
        for i, (o, s) in enumerate(CH):
            place(pack, f"wrzh{i}", wrzh[o:o + s])
            place(pack, f"whn{i}", whn[o:o + s])
            place(pack, f"wg{i}", wgh[o:o + s])
            place(pack, f"wm{i}", wmh[o:o + s])
            place(pack, f"w12{i}", w12[o:o + s])
        place(pack, "wrzx", wrzx)
        wfix = np.zeros((128, FIXCOLS), dtype=f)
        wfix[:, 0:128] = ident
        adjg = adjc.reshape(bs, MAXN * MAXN)
        wfix[:, 128:128 + MAXN * MAXN] = adjg[:128]
        wfix[:, 128 + MAXN * MAXN:] = adjg[128:]

        constt = np.zeros((MAXN * 2 * 128, HS2), dtype=f)
        try:
            import ml_dtypes
            bf = ml_dtypes.bfloat16
        except ImportError:
            bf = None
        gint = np.zeros((MAXN * 2 * 128, HS2),
                        dtype=(bf if bf is not None else f))
        cc = const[slc]                               # [bs, 20, 501]
        gc = ginb[slc]
        for v in range(MAXN):
            for t in range(2):
                r0 = (v * 2 + t) * 128
                constt[r0:r0 + 128, :HS] = cc[t * 128:(t + 1) * 128, v]
                constt[r0:r0 + 128, HS] = 1.0        # ones col -> bias rows
                gint[r0:r0 + 128, :HS] = gc[t * 128:(t + 1) * 128, v]
        in_maps.append(dict(wpack=pack, wfix=wfix, constt=constt, gint=gint,
                            wgvb=wgvb.astype(bfdt), wmvb=wmvb.astype(bfdt)))
    return in_maps


def _get_prog():
    global _PROG
    if _PROG is None:
        _PROG = _build_program()
    return _PROG


def kernel(**inputs):
    from concourse.bass_utils import run_bass_kernel_spmd
    nc = _get_prog()
    in_maps = _host_prep(**inputs)
    res = run_bass_kernel_spmd(nc, in_maps, core_ids=list(range(NCORES)))
    out = np.concatenate([r["out"] for r in res.results], axis=0)
    mu = np.ascontiguousarray(out[:, :NZ])
    logvar = np.ascontiguousarray(out[:, NZ:])
    return mu, logvar
